# revision 1
# baseline (speedup 1.0000x reference)
"""TRN2 Bass kernel for nn_AttentionModule (dense transformer attention block).

Reference computation (per sample b, x flattened to [256, 4096]):
    proj = conv_w @ x + conv_b                 [32, 4096]
    q    = (q_w @ proj + q_b).T                [4096, 32]
    k    = k_w @ proj + k_b                    [32, 4096]
    v    = v_w @ proj + v_b                    [256, 4096]
    attn = softmax(q @ k, axis=-1)             [4096(n), 4096(m)]
    out  = gamma * (v @ attn.T) + x            [256, 4096]

Sharding: 8 cores = 4 samples x 2 query-halves (2048 queries each). Each core
redundantly computes proj/k/v for its sample (cheap) and its half of the
queries. No cross-core communication. SPMD: odd cores receive x with the
spatial axis rolled by -2048 so "their" queries sit at columns 0:2048;
attention is permutation-invariant over keys so k/v column order is free.

On-core layout: scores are computed transposed, [m_keys(part), n_queries
(free)], so the exp'd scores chunks are directly usable as matmul weights
(lhsT) for the attn@V contraction over m, and the softmax denominator falls
out of the same matmul via an appended ones-column in the V^T projection
(column 256 of the [33,257] rhs; proj carries a ones-row 32 that also folds
in the v bias). No max-subtraction: exp'd scores are stored in bf16 (no
overflow below e^88); numerator and denominator share the same bf16 rounding
so softmax normalization cancels most of it. The residual is applied in
[n, c] layout against a host-transposed x, and the host transposes the
[2048, 256] per-core output back — no on-chip transposes at all.

gamma is folded into v_w/v_b host-side. fp16 feeds the q/k score path.
"""

import numpy as np
from contextlib import ExitStack

import concourse.bass as bass
import concourse.bacc as bacc
import concourse.tile as tile
from concourse import mybir
from concourse.bass_utils import run_bass_kernel_spmd

F32 = mybir.dt.float32
F16 = mybir.dt.float16
BF16 = mybir.dt.bfloat16

B, C, H, W = 4, 256, 64, 64
HW = H * W          # 4096 keys (m)
NQ = HW // 2        # 2048 queries per core (n)
C8 = 32             # qk head dim (e) / proj channels (d)
NSUP = 512          # queries per attention super-block
NBLK = 128          # queries per attnout block
MCH = 128           # keys per m-chunk (one lhsT tile)
N_MCH = HW // MCH   # 32 m-chunks
VN = C + 1          # 257: v channels + ones column (softmax denominator)

_CACHED = {}


def build_nc():
    nc = bacc.Bacc("TRN2", target_bir_lowering=False, debug=False)
    d_x16 = nc.dram_tensor("x16", [C, HW], F16, kind="ExternalInput").ap()
    d_xT = nc.dram_tensor("xT", [NQ, C], F32, kind="ExternalInput").ap()
    d_cwT = nc.dram_tensor("cwT", [2, 128, C8], F16, kind="ExternalInput").ap()
    d_cb = nc.dram_tensor("cb", [C8, 1], F32, kind="ExternalInput").ap()
    # k/q weights carry their bias as row 32, contracted against proj's
    # ones-row — no separate bias op needed.
    d_kwT = nc.dram_tensor("kwT", [C8 + 1, C8], F16, kind="ExternalInput").ap()
    d_qwT = nc.dram_tensor("qwT", [C8 + 1, C8], F16, kind="ExternalInput").ap()
    d_vwb = nc.dram_tensor("vwb", [C8 + 1, VN], F16, kind="ExternalInput").ap()
    d_outT = nc.dram_tensor("outT", [NQ, C], F32, kind="ExternalOutput").ap()

    with tile.TileContext(nc) as tc, ExitStack() as ctx:
        const_pool = ctx.enter_context(tc.tile_pool(name="const", bufs=1))
        big_pool = ctx.enter_context(tc.tile_pool(name="big", bufs=1))

        # ---- constants / inputs ----
        cwT = const_pool.tile([128, 2, C8], F16)
        kwT = const_pool.tile([C8 + 1, C8], F16)
        qwT = const_pool.tile([C8 + 1, C8], F16)
        vwb = const_pool.tile([C8 + 1, VN], F16)
        cb = const_pool.tile([C8, 1], F32)
        warm = const_pool.tile([128, 512], F16)
        for a in range(2):
            nc.sync.dma_start(cwT[:, a, :], d_cwT[a])
        nc.sync.dma_start(kwT[:], d_kwT)
        nc.sync.dma_start(qwT[:], d_qwT)
        nc.sync.dma_start(vwb[:], d_vwb)
        nc.sync.dma_start(cb[:], d_cb)
        nc.gpsimd.memset(warm[:], 0.0)

        # x16: two c-halves [128, HW] fp16 (matmul operand); DMA in fine
        # chunks interleaved across two HWDGE queues so the first proj
        # matmul can start as soon as the first column chunk lands.
        x16 = [big_pool.tile([128, HW], F16, tag=f"x16_{i}", name=f"x16_{i}") for i in range(2)]
        d_x16v = d_x16.rearrange("(a p) m -> a p m", p=128)
        for j in range(8):
            for i in range(2):
                sl = bass.ts(j, HW // 8)
                eng = nc.sync if i == 0 else nc.scalar
                eng.dma_start(x16[i][:, sl], d_x16v[i][:, sl])

        # xT: residual input, [128, nb, 256]: query block nb on partitions.
        # On the gpsimd DMA queue: not needed until the attnout epilogue.
        xT = big_pool.tile([128, NQ // NBLK, C], F32)
        d_xTv = d_xT.rearrange("(nb p) c -> p nb c", p=128)
        for j in range(4):
            nbs = NQ // NBLK // 4
            nc.gpsimd.dma_start(xT[:, j * nbs : (j + 1) * nbs, :],
                                d_xTv[:, j * nbs : (j + 1) * nbs, :])

        proj = big_pool.tile([C8 + 1, HW], F16)   # row 32 = ones
        nc.gpsimd.memset(proj[C8 : C8 + 1, :], 1.0)
        k4 = big_pool.tile([128, HW], F16)        # k replicated on 4 row-groups
        qT4 = big_pool.tile([128, NQ], F16)       # query half, replicated x4
        vt = big_pool.tile([128, N_MCH * VN], BF16)  # vT' chunks [m=128, 257]

        # ---- the PSUM ring ----
        # ALL psum flows through one ring of 2 slots x 4 banks: projection
        # slices, score groups (so exp reads 2048-wide APs: the ACT per-op
        # drain tax is paid 32x, not 64x), vT' quads, attnout accumulators.
        ring = ctx.enter_context(tc.tile_pool(name="ring", bufs=2, space="PSUM"))
        att_pool = ctx.enter_context(tc.tile_pool(name="att", bufs=2))
        out_pool = ctx.enter_context(tc.tile_pool(name="outp", bufs=3))

        def rtile(shape, name):
            return ring.tile(shape, F32, tag="ps", name=name)

        SL = 2048                          # psum slice width (4 banks fp32)

        # PE warmup: dummy matmuls on zeros while the input DMAs land, so
        # the HAM clock-gate is released before the real work starts.
        pw = rtile([C8, SL], "pw")
        for _ in range(36):
            nc.tensor.matmul(pw[:, 0:512], cwT[:, 0, :], warm[:])

        # proj = conv_w @ x + conv_b  (K = 256 over 2 chunks), bias by ACT
        for s in range(HW // SL):
            pp = rtile([C8, SL], f"pp{s}")
            for jj in range(4):
                sl = bass.ts(jj, 512)
                gsl = bass.ds(s * SL + jj * 512, 512)
                nc.tensor.matmul(pp[:, sl], cwT[:, 0, :], x16[0][:, gsl],
                                 start=True, stop=False)
                nc.tensor.matmul(pp[:, sl], cwT[:, 1, :], x16[1][:, gsl],
                                 start=False, stop=True)
            nc.scalar.activation(
                proj[0:C8, bass.ds(s * SL, SL)], pp[:],
                mybir.ActivationFunctionType.Identity, bias=cb[:])

        # qT4 = q_w' @ proj' (bias via proj ones-row), x4 col-groups
        pq = rtile([128, NQ], "pq")
        for jj in range(4):
            sl = bass.ts(jj, 512)
            for g in range(4):
                nc.tensor.matmul(pq[bass.ts(g, 32), sl], qwT[:],
                                 proj[:, sl], tile_position=(0, 32 * g))
        for h in range(2):
            nc.vector.tensor_copy(qT4[:, bass.ts(h, NQ // 2)],
                                  pq[:, bass.ts(h, NQ // 2)])

        # k4 = k_w' @ proj' on all 4 col-groups (x4 replication)
        for s in range(HW // SL):
            pk = rtile([128, SL], f"pk{s}")
            for jj in range(4):
                sl = bass.ts(jj, 512)
                gsl = bass.ds(s * SL + jj * 512, 512)
                for g in range(4):
                    nc.tensor.matmul(pk[bass.ts(g, 32), sl], kwT[:],
                                     proj[:, gsl], tile_position=(0, 32 * g))
            if s == 0:
                nc.vector.tensor_copy(k4[:, bass.ds(s * SL, SL)], pk[:])
            else:
                nc.scalar.copy(k4[:, bass.ds(s * SL, SL)], pk[:])

        # ---- attention ----
        n_sup = NQ // NSUP                # 4 super-blocks of 512 queries
        n_blk = NSUP // NBLK              # 4 attnout blocks per super
        GCH = 4                           # m-chunks per scores group (4 banks)
        n_grp = N_MCH // GCH              # 8 scores groups per super
        e_sbs = {}

        def alloc_e(ns):
            e_sbs[ns] = att_pool.tile([128, N_MCH * NSUP], BF16, tag="e_sb",
                                      name=f"e_sb_{ns}")

        def emit_score_group(ns, g):
            nsl = bass.ts(ns, NSUP)
            e_sb = e_sbs[ns]
            ps = rtile([128, GCH * NSUP], f"ps_{ns}_{g}")
            for i in range(GCH):
                mi = GCH * g + i
                nc.tensor.matmul(
                    ps[:, bass.ts(i, NSUP)],
                    k4[bass.ts(i, 32), bass.ts(mi, MCH)],
                    qT4[bass.ts(i, 32), nsl],
                    tile_position=(32 * i, 0),
                )
            nc.scalar.activation(
                e_sb[:, bass.ds(GCH * g * NSUP, GCH * NSUP)], ps[:],
                mybir.ActivationFunctionType.Exp)

        # scores + exp for super 0 run interleaved with the vT' build: the
        # exp stream paces ACT while vT' copies ride the otherwise-idle DVE.
        alloc_e(0)
        for g in range(n_grp):
            emit_score_group(0, g)
            pv = rtile([128, 4, 512], f"pv{g}")
            for i in range(4):
                mi = 4 * g + i
                nc.tensor.matmul(pv[:, i, 0:VN], proj[:, bass.ts(mi, MCH)],
                                 vwb[:])
            vt_sl = vt[:, bass.ds(4 * g * VN, 4 * VN)].rearrange(
                "p (a v) -> p a v", v=VN)
            nc.vector.tensor_copy(vt_sl, pv[:, :, 0:VN])

        def emit_block_epilogue(po, nbg):
            rcol = out_pool.tile([128, 1], F32, tag="rcol",
                                 name=f"rcol_{nbg}")
            nc.vector.reciprocal(rcol[:], po[:, C : C + 1])
            anorm = out_pool.tile([128, C], F32, tag="anorm",
                                  name=f"anorm_{nbg}")
            nc.vector.tensor_scalar_mul(anorm[:], po[:, 0:C], rcol[:])
            osb = out_pool.tile([128, C], F32, tag="osb", name=f"osb_{nbg}")
            nc.vector.tensor_add(osb[:], anorm[:], xT[:, nbg, :])
            nc.sync.dma_start(
                d_outT.rearrange("(nb p) c -> p nb c", p=128)[:, nbg, :],
                osb[:])

        def emit_attnout_block(ns, nb):
            e_sb = e_sbs[ns]
            po = rtile([128, VN], f"po_{ns}_{nb}")
            for mi in range(N_MCH):
                nc.tensor.matmul(
                    po[:],
                    e_sb[:, bass.ds(mi * NSUP + nb * NBLK, NBLK)],
                    vt[:, bass.ts(mi, VN)],
                    start=(mi == 0), stop=(mi == N_MCH - 1),
                )
            emit_block_epilogue(po, ns * n_blk + nb)

        def emit_attnout_pair(ns, nbs):
            # interleave two blocks' accumulation chains chunk-by-chunk
            e_sb = e_sbs[ns]
            pos = [rtile([128, VN], f"pot_{ns}_{nb}") for nb in nbs]
            for mi in range(N_MCH):
                for po, nb in zip(pos, nbs):
                    nc.tensor.matmul(
                        po[:],
                        e_sb[:, bass.ds(mi * NSUP + nb * NBLK, NBLK)],
                        vt[:, bass.ts(mi, VN)],
                        start=(mi == 0), stop=(mi == N_MCH - 1),
                    )
            for po, nb in zip(pos, nbs):
                emit_block_epilogue(po, ns * n_blk + nb)

        # Steady state: per attnout block of super S, two scores groups of
        # super S+1 around it — ACT (exp, 2us/op) stays fed at the PE's
        # block rate (~4us) with no psum-slot stalls. The final super has
        # no successor scores, so its blocks run as interleaved pairs that
        # track the tail of the exp stream.
        for ns in range(n_sup):
            if ns + 1 < n_sup:
                alloc_e(ns + 1)
                for nb in range(n_blk):
                    emit_score_group(ns + 1, 2 * nb)
                    emit_attnout_block(ns, nb)
                    emit_score_group(ns + 1, 2 * nb + 1)
            else:
                emit_attnout_pair(ns, [0, 1])
                emit_attnout_pair(ns, [2, 3])
            e_sbs.pop(ns)

    nc.compile()
    return nc


def _prep_in_maps(x, conv_w, conv_b, q_w, q_b, k_w, k_b, v_w, v_b, gamma):
    g = np.float32(gamma[0])
    cwT = np.ascontiguousarray(conv_w.T.reshape(2, 128, C8)).astype(np.float16)
    kwT = np.concatenate([k_w.T, k_b[None, :]], axis=0).astype(np.float16)
    qwT = np.concatenate([q_w.T, q_b[None, :]], axis=0).astype(np.float16)
    vwb = np.zeros((C8 + 1, VN), np.float16)
    vwb[0:C8, 0:C] = (g * v_w).T.astype(np.float16)
    vwb[C8, 0:C] = (g * v_b).astype(np.float16)
    vwb[C8, C] = 1.0
    cb = conv_b.reshape(C8, 1).astype(np.float32)

    in_maps = []
    for core in range(8):
        b, hf = core // 2, core % 2
        xf = np.asarray(x[b], np.float32).reshape(C, HW)
        if hf:
            # rotate spatial columns: this core's query half -> cols 0:2048
            xf = np.roll(xf, -NQ, axis=1)
        in_maps.append({
            "x16": np.ascontiguousarray(xf).astype(np.float16),
            "xT": np.ascontiguousarray(xf[:, 0:NQ].T),
            "cwT": cwT, "cb": cb, "kwT": kwT, "qwT": qwT, "vwb": vwb,
        })
    return in_maps


def kernel(x, conv_w, conv_b, q_w, q_b, k_w, k_b, v_w, v_b, gamma, **run_kw):
    if "nc" not in _CACHED:
        _CACHED["nc"] = build_nc()
    nc = _CACHED["nc"]
    in_maps = _prep_in_maps(x, conv_w, conv_b, q_w, q_b, k_w, k_b, v_w, v_b,
                            gamma)
    res = run_bass_kernel_spmd(nc, in_maps, core_ids=list(range(8)), **run_kw)
    _CACHED["last_result"] = res
    out = np.empty((B, C, HW), np.float32)
    for core in range(8):
        b, hf = core // 2, core % 2
        oc = np.asarray(res.results[core]["outT"])  # [2048, 256]
        out[b, :, hf * NQ : (hf + 1) * NQ] = oc.T
    return out.reshape(B, C, H, W)

